# revision 13
# baseline (speedup 1.0000x reference)
"""Soft-KMeans (retrieval_knn) Trainium2 kernel, 8-core SPMD.

Full inputs in, full output (scalar accuracy) out. Internally:
 - host: support-set preprocessing (tiny: 1000x1024), query sharding 8-way
 - device: query normalization, n_iter soft-kmeans iterations with an
   AllReduce of per-class weighted sums each iteration, final argmax
   accuracy count.

Data layout: normalized queries are stored bf16 in DRAM twice --
natural [NQS, D] (contiguous rhs loads for the w^T q matmul) and
chunk-major [D/128, NQS, 128] (contiguous sources for the xbar
transpose loads feeding the q @ c^T scores matmul).
"""
import os
import numpy as np
import ml_dtypes

import concourse.bass as bass
import concourse.bacc as bacc
import concourse.mybir as mybir
import concourse.tile as tile
from concourse.bass_utils import run_bass_kernel_spmd

F32 = mybir.dt.float32
BF16 = mybir.dt.bfloat16
AF = mybir.ActivationFunctionType
OP = mybir.AluOpType

N_CORES = 8
NQ = 131072
D = 1024
C = 100
P = 128
NQS = NQ // N_CORES          # queries per core (16384)
KSUB = 4                     # 128-row sub-tiles per super-tile
SUP = P * KSUB               # 512 queries per super-tile
NSUP = NQS // SUP            # super-tiles per core (32)
DCH = D // P                 # d chunks (8)

_BUILD_CACHE = {}
LAST_EXEC_NS = None


def ts(i, n):
    return slice(i * n, (i + 1) * n)


def build(n_iter: int, beta: float):
    key = (n_iter, float(beta))
    if key in _BUILD_CACHE:
        return _BUILD_CACHE[key]

    two_beta = 2.0 * float(beta)

    nc = bacc.Bacc(None, target_bir_lowering=False, num_devices=N_CORES)

    qx_d = nc.dram_tensor("qx", [NQS, D], F32, kind="ExternalInput")
    y1h_d = nc.dram_tensor("y1h", [NQS, C], F32, kind="ExternalInput")
    mean_d = nc.dram_tensor("mean", [1, D], F32, kind="ExternalInput")
    ssum_d = nc.dram_tensor("ssum", [C, D], F32, kind="ExternalInput")
    scnt_d = nc.dram_tensor("scnt", [C, 1], F32, kind="ExternalInput")
    c0t_d = nc.dram_tensor("c0t", [P, DCH * C], BF16, kind="ExternalInput")
    b0_d = nc.dram_tensor("b0", [1, C], F32, kind="ExternalInput")
    ident_d = nc.dram_tensor("ident", [P, P], F32, kind="ExternalInput")
    ones_d = nc.dram_tensor("ones", [P, 1], BF16, kind="ExternalInput")
    # zmax - z[y] per query; exact 0.0 <=> prediction correct (host counts)
    acc_d = nc.dram_tensor("acc_out", [P, NQS // P], F32, kind="ExternalOutput")

    with tile.TileContext(nc) as tc:
        with (
            tc.tile_pool(name="pers", bufs=1) as pers,
            tc.tile_pool(name="dram", bufs=1, space="DRAM") as dram,
            tc.tile_pool(name="ccp", bufs=2, space="DRAM") as ccp,
            tc.tile_pool(name="ps_sc", bufs=3, space="PSUM") as ps_sc,
            tc.tile_pool(name="ps_a", bufs=1, space="PSUM") as ps_a,
            tc.tile_pool(name="ps_b", bufs=1, space="PSUM") as ps_b,
            tc.tile_pool(name="ps_t", bufs=1, space="PSUM") as ps_t,
        ):
            # ---------------- persistent tiles ----------------
            mean_bc = pers.tile([P, D], F32)
            nc.sync.dma_start(mean_bc[:], mean_d[:].broadcast_to([P, D]))

            ssum_sb = pers.tile([C, D], F32)
            nc.sync.dma_start(ssum_sb[:], ssum_d[:])
            scnt_sb = pers.tile([C, 1], F32)
            nc.sync.dma_start(scnt_sb[:], scnt_d[:])

            cT_sb = pers.tile([P, DCH * C], BF16)
            nc.sync.dma_start(cT_sb[:], c0t_d[:])

            Bb = pers.tile([P, C], F32)
            nc.sync.dma_start(Bb[:], b0_d[:].broadcast_to([P, C]))

            ident_sb = pers.tile([P, P], F32)
            nc.sync.dma_start(ident_sb[:], ident_d[:])
            ones_sb = pers.tile([P, 1], BF16)
            nc.sync.dma_start(ones_sb[:], ones_d[:])

            matches = pers.tile([P, NQS // P], F32)
            cent = pers.tile([C, D], F32)
            csq = pers.tile([C, D], F32)
            red_sb = pers.tile([C, D + 1], BF16)
            redr_sb = pers.tile([C, D + 1], BF16)
            b_row = pers.tile([1, C], F32)
            cnt_t = pers.tile([C, 1], F32)
            rc_t = pers.tile([C, 1], F32)
            bc2 = pers.tile([C, 1], F32)

            q_bf = dram.tile([NQS, D], BF16)          # natural layout
            q_cm = dram.tile([DCH, NQS, P], BF16)     # chunk-major layout
            b_dram = dram.tile([1, C], F32)

            # ---------------- pass 1: normalize queries ----------------
            with tc.tile_pool(name="p1", bufs=1) as p1:
                for s in range(NSUP):
                    qx_sup = p1.tile([P, KSUB, D], F32, tag="qx", bufs=2)
                    nc.scalar.dma_start(
                        qx_sup[:],
                        qx_d[ts(s, SUP), :].rearrange("(k p) d -> p k d", p=P),
                    )
                    for k in range(KSUB):
                        t = s * KSUB + k
                        qc = p1.tile([P, D], F32, tag="qc", bufs=2)
                        nc.vector.scalar_tensor_tensor(
                            qc[:], qx_sup[:, k, :], 1.0, mean_bc[:],
                            op0=OP.mult, op1=OP.subtract,
                        )
                        sq = p1.tile([P, D], F32, tag="sq", bufs=2)
                        nrm = p1.tile([P, 1], F32, tag="nrm", bufs=2)
                        nc.scalar.activation(sq[:], qc[:], AF.Square,
                                             accum_out=nrm[:])
                        rin = p1.tile([P, 1], F32, tag="rin", bufs=2)
                        nc.vector.reciprocal(rin[:], nrm[:])
                        rs = p1.tile([P, 1], F32, tag="rs", bufs=2)
                        nc.scalar.activation(rs[:], rin[:], AF.Sqrt)
                        qb = p1.tile([P, D], BF16, tag="qb", bufs=3)
                        nc.vector.tensor_scalar(
                            qb[:], qc[:], rs[:], None, op0=OP.mult
                        )
                        nc.sync.dma_start(q_bf[ts(t, P), :], qb[:])
                        for j in range(DCH):
                            nc.sync.dma_start(
                                q_cm[j, ts(t, P), :], qb[:, ts(j, P)]
                            )

            # ---------------- iteration + pred passes ----------------
            with (
                tc.tile_pool(name="pq", bufs=3) as pq,
                tc.tile_pool(name="pt", bufs=3) as pt,
                tc.tile_pool(name="pw", bufs=3) as pw,
                tc.tile_pool(name="py", bufs=2) as py,
            ):
                for it in range(n_iter + 1):
                    pred = it == n_iter
                    if not pred:
                        suma_ps = ps_a.tile([C, D], F32, tag="suma")
                        sumb_ps = ps_b.tile([C, 1], F32, tag="sumb")

                    for s in range(NSUP):
                        qT_sup = pt.tile([P, DCH, SUP], BF16, tag="qT")
                        for j in range(DCH):
                            nc.sync.dma_start(
                                qT_sup[:, j, :],
                                q_cm[j, ts(s, SUP), :],
                                transpose=True,
                            )
                        if not pred:
                            q_sup = pq.tile([P, KSUB, D], BF16, tag="q")
                            nc.scalar.dma_start(
                                q_sup[:],
                                q_bf[ts(s, SUP), :].rearrange(
                                    "(k p) d -> p k d", p=P
                                ),
                            )
                        else:
                            yh_sup = py.tile([P, KSUB, C], F32, tag="yh")
                            nc.scalar.dma_start(
                                yh_sup[:],
                                y1h_d[ts(s, SUP), :].rearrange(
                                    "(k p) c -> p k c", p=P
                                ),
                            )

                        for k in range(KSUB):
                            sc = ps_sc.tile([P, C], F32, tag="sc")
                            for j in range(DCH):
                                nc.tensor.matmul(
                                    sc[:],
                                    qT_sup[:, j, ts(k, P)],
                                    cT_sb[:, ts(j, C)],
                                    start=(j == 0),
                                    stop=(j == DCH - 1),
                                )
                            zt = pw.tile([P, C], F32, tag="zt")
                            nc.vector.scalar_tensor_tensor(
                                zt[:], sc[:], two_beta, Bb[:],
                                op0=OP.mult, op1=OP.subtract,
                            )
                            if not pred:
                                wt = pw.tile([P, C], BF16, tag="wt")
                                rsum = pw.tile([P, 1], F32, tag="rsum")
                                nc.scalar.activation(
                                    wt[:], zt[:], AF.Exp, accum_out=rsum[:]
                                )
                                rcp = pw.tile([P, 1], F32, tag="rcp")
                                nc.vector.reciprocal(rcp[:], rsum[:])
                                wn = pw.tile([P, C], BF16, tag="wn")
                                nc.scalar.mul(wn[:], wt[:], rcp[:])

                                first = (s == 0 and k == 0)
                                last = (s == NSUP - 1 and k == KSUB - 1)
                                nc.tensor.matmul(
                                    suma_ps[:, 0:512], wn[:],
                                    q_sup[:, k, 0:512],
                                    start=first, stop=last,
                                )
                                nc.tensor.matmul(
                                    suma_ps[:, 512:1024], wn[:],
                                    q_sup[:, k, 512:1024],
                                    start=first, stop=last,
                                )
                                nc.tensor.matmul(
                                    sumb_ps[:], wn[:], ones_sb[:],
                                    start=first, stop=last,
                                )
                            else:
                                zmax = pw.tile([P, 1], F32, tag="zmax")
                                nc.vector.reduce_max(
                                    zmax[:], zt[:], axis=mybir.AxisListType.X
                                )
                                zsc = pw.tile([P, C], F32, tag="zsc")
                                zy = pw.tile([P, 1], F32, tag="zy")
                                nc.vector.scalar_tensor_tensor(
                                    zsc[:], zt[:], 1.0, yh_sup[:, k, :],
                                    op0=OP.mult, op1=OP.mult,
                                    accum_out=zy[:],
                                )
                                t_idx = s * KSUB + k
                                nc.vector.scalar_tensor_tensor(
                                    matches[:, t_idx:t_idx + 1],
                                    zmax[:], 1.0, zy[:],
                                    op0=OP.mult, op1=OP.subtract,
                                )

                    if pred:
                        nc.sync.dma_start(acc_d[:], matches[:])
                        continue

                    # ------- iteration update: allreduce + new centroids -----
                    nc.scalar.copy(red_sb[:, 0:D], suma_ps[:])
                    nc.scalar.copy(red_sb[:, D:D + 1], sumb_ps[:])
                    cc_in = ccp.tile([C, D + 1], BF16, tag="cci")
                    cc_out = ccp.tile([C, D + 1], BF16, tag="cco",
                                      addr_space="Shared")
                    nc.scalar.dma_start(cc_in[:], red_sb[:])
                    nc.gpsimd.collective_compute(
                        "AllReduce",
                        OP.add,
                        replica_groups=[list(range(N_CORES))],
                        ins=[cc_in[:].opt()],
                        outs=[cc_out[:].opt()],
                    )
                    nc.scalar.dma_start(redr_sb[:], cc_out[:])

                    nc.vector.scalar_tensor_tensor(
                        cent[:], redr_sb[:, 0:D], 1.0, ssum_sb[:],
                        op0=OP.mult, op1=OP.add,
                    )
                    nc.vector.scalar_tensor_tensor(
                        cnt_t[:], redr_sb[:, D:D + 1], 1.0, scnt_sb[:],
                        op0=OP.mult, op1=OP.add,
                    )
                    nc.vector.reciprocal(rc_t[:], cnt_t[:])
                    nc.vector.tensor_scalar(
                        cent[:], cent[:], rc_t[:], None, op0=OP.mult
                    )
                    nc.vector.scalar_tensor_tensor(
                        csq[:], cent[:], float(beta), cent[:],
                        op0=OP.mult, op1=OP.mult,
                        accum_out=bc2[:],
                    )
                    for j in range(DCH):
                        ct_ps = ps_t.tile([P, C], F32, tag="ct")
                        nc.tensor.transpose(
                            ct_ps[:], cent[:, ts(j, P)], ident_sb[0:C, 0:C]
                        )
                        nc.scalar.copy(cT_sb[:, ts(j, C)], ct_ps[:])
                    b_ps = ps_t.tile([1, C], F32, tag="bt")
                    nc.tensor.transpose(b_ps[:], bc2[:], ident_sb[0:C, 0:C])
                    nc.scalar.copy(b_row[:], b_ps[:])
                    nc.scalar.dma_start(b_dram[:], b_row[:])
                    nc.scalar.dma_start(
                        Bb[:], b_dram[:].broadcast_to([P, C])
                    )

    nc.compile()
    _BUILD_CACHE[key] = nc
    return nc


def kernel(**inputs) -> np.ndarray:
    global LAST_EXEC_NS
    support_x = np.asarray(inputs["support_x"], dtype=np.float64)
    support_y = np.asarray(inputs["support_y"]).astype(np.int64)
    query_x = np.asarray(inputs["query_x"], dtype=np.float32)
    query_y = np.asarray(inputs["query_y"]).astype(np.int64)
    n_iter = int(inputs["n_iter"])
    beta = float(inputs["beta"])
    num_classes = int(inputs["num_classes"])
    assert num_classes == C and query_x.shape == (NQ, D)
    assert support_x.shape == (len(support_y), D)

    # ---- host: support-set preprocessing (tiny) ----
    mean = support_x.mean(axis=0)
    s = support_x - mean
    s /= np.linalg.norm(s, axis=1, keepdims=True)
    sup_sum = np.zeros((C, D), dtype=np.float64)
    np.add.at(sup_sum, support_y, s)
    sup_cnt = np.bincount(support_y, minlength=C).astype(np.float64)
    c0 = sup_sum / sup_cnt[:, None]
    b0 = beta * (c0 * c0).sum(axis=1)

    y1h = np.zeros((NQ, C), dtype=np.float32)
    y1h[np.arange(NQ), query_y] = 1.0

    nc = build(n_iter, beta)

    mean_f = mean.astype(np.float32)[None, :]
    ssum_f = sup_sum.astype(np.float32)
    scnt_f = sup_cnt.astype(np.float32)[:, None]
    # [P, DCH*C]: row p holds [c0T[j*P+p, c] for j in range(DCH) for c in range(C)]
    c0t = np.ascontiguousarray(
        c0.T.reshape(DCH, P, C).transpose(1, 0, 2).reshape(P, DCH * C)
    ).astype(ml_dtypes.bfloat16)
    b0_f = b0.astype(np.float32)[None, :]
    ident = np.eye(P, dtype=np.float32)
    ones = np.ones((P, 1), dtype=ml_dtypes.bfloat16)

    in_maps = []
    for i in range(N_CORES):
        in_maps.append({
            "qx": np.ascontiguousarray(query_x[ts(i, NQS)]),
            "y1h": np.ascontiguousarray(y1h[ts(i, NQS)]),
            "mean": mean_f,
            "ssum": ssum_f,
            "scnt": scnt_f,
            "c0t": c0t,
            "b0": b0_f,
            "ident": ident,
            "ones": ones,
        })

    trace = bool(int(os.environ.get("KM_TRACE", "0")))
    if trace:
        import profile_util
        results, outdir = profile_util.profile_run(nc, in_maps, N_CORES)
        print(f"profile dir: {outdir}")
        try:
            LAST_EXEC_NS, _ = profile_util.exec_time_ns(outdir, device=0)
        except Exception as e:
            print(f"profile post-processing failed: {e}")
    else:
        res = run_bass_kernel_spmd(
            nc, in_maps, core_ids=list(range(N_CORES))
        )
        results = res.results
    total = sum(int((r["acc_out"] == 0.0).sum()) for r in results)
    return np.float32(total / NQ)


# revision 14
# speedup vs baseline: 1.9202x; 1.9202x over previous
"""Soft-KMeans (retrieval_knn) Trainium2 kernel, 8-core SPMD.

Full inputs in, full output (scalar accuracy) out. Internally:
 - host: support-set preprocessing (tiny: 1000x1024), query sharding 8-way
 - device: query normalization, n_iter soft-kmeans iterations with an
   AllReduce of per-class weighted sums each iteration, final argmax
   accuracy count.

Data layout: normalized queries are stored bf16 in DRAM twice --
natural [NQS, D] (rhs of the w^T q matmul; per-128-row blocks are
address-sequential) and transposed pre-tiled [NSUP, D/128, 128, 512]
(lhsT of the q @ c^T scores matmul; per-(super, chunk) blocks are
address-sequential). The transpose itself runs once, on the PE during
pass 1 -- xbar transpose DMAs from HBM move 256B packets and measure
~80 GB/s, so they are avoided entirely in the iteration loop.
"""
import os
import numpy as np
import ml_dtypes

import concourse.bass as bass
import concourse.bacc as bacc
import concourse.mybir as mybir
import concourse.tile as tile
from concourse.bass_utils import run_bass_kernel_spmd

F32 = mybir.dt.float32
BF16 = mybir.dt.bfloat16
AF = mybir.ActivationFunctionType
OP = mybir.AluOpType

N_CORES = 8
NQ = 131072
D = 1024
C = 100
P = 128
NQS = NQ // N_CORES          # queries per core (16384)
KSUB = 4                     # 128-row sub-tiles per super-tile
SUP = P * KSUB               # 512 queries per super-tile
NSUP = NQS // SUP            # super-tiles per core (32)
DCH = D // P                 # d chunks (8)

_BUILD_CACHE = {}
LAST_EXEC_NS = None


def ts(i, n):
    return slice(i * n, (i + 1) * n)


def build(n_iter: int, beta: float):
    key = (n_iter, float(beta))
    if key in _BUILD_CACHE:
        return _BUILD_CACHE[key]

    two_beta = 2.0 * float(beta)

    nc = bacc.Bacc(None, target_bir_lowering=False, num_devices=N_CORES)

    qx_d = nc.dram_tensor("qx", [NQS, D], F32, kind="ExternalInput")
    y1h_d = nc.dram_tensor("y1h", [NQS, C], F32, kind="ExternalInput")
    mean_d = nc.dram_tensor("mean", [1, D], F32, kind="ExternalInput")
    ssum_d = nc.dram_tensor("ssum", [C, D], F32, kind="ExternalInput")
    scnt_d = nc.dram_tensor("scnt", [C, 1], F32, kind="ExternalInput")
    c0t_d = nc.dram_tensor("c0t", [P, DCH * C], BF16, kind="ExternalInput")
    b0_d = nc.dram_tensor("b0", [1, C], F32, kind="ExternalInput")
    ident_d = nc.dram_tensor("ident", [P, P], F32, kind="ExternalInput")
    identb_d = nc.dram_tensor("identb", [P, P], BF16, kind="ExternalInput")
    ones_d = nc.dram_tensor("ones", [P, 1], BF16, kind="ExternalInput")
    # zmax - z[y] per query; exact 0.0 <=> prediction correct (host counts)
    acc_d = nc.dram_tensor("acc_out", [P, NQS // P], F32, kind="ExternalOutput")

    with tile.TileContext(nc) as tc:
        with (
            tc.tile_pool(name="pers", bufs=1) as pers,
            tc.tile_pool(name="dram", bufs=1, space="DRAM") as dram,
            tc.tile_pool(name="ccp", bufs=2, space="DRAM") as ccp,
            tc.tile_pool(name="ps_sc", bufs=3, space="PSUM") as ps_sc,
            tc.tile_pool(name="ps_a", bufs=1, space="PSUM") as ps_a,
            tc.tile_pool(name="ps_b", bufs=1, space="PSUM") as ps_b,
            tc.tile_pool(name="ps_t", bufs=1, space="PSUM") as ps_t,
        ):
            # ---------------- persistent tiles ----------------
            mean_bc = pers.tile([P, D], F32)
            nc.sync.dma_start(mean_bc[:], mean_d[:].broadcast_to([P, D]))

            ssum_sb = pers.tile([C, D], F32)
            nc.sync.dma_start(ssum_sb[:], ssum_d[:])
            scnt_sb = pers.tile([C, 1], F32)
            nc.sync.dma_start(scnt_sb[:], scnt_d[:])

            cT_sb = pers.tile([P, DCH * C], BF16)
            nc.sync.dma_start(cT_sb[:], c0t_d[:])

            Bb = pers.tile([P, C], F32)
            nc.sync.dma_start(Bb[:], b0_d[:].broadcast_to([P, C]))

            ident_sb = pers.tile([P, P], F32)
            nc.sync.dma_start(ident_sb[:], ident_d[:])
            identb_sb = pers.tile([P, P], BF16)
            nc.sync.dma_start(identb_sb[:], identb_d[:])
            ones_sb = pers.tile([P, 1], BF16)
            nc.sync.dma_start(ones_sb[:], ones_d[:])

            matches = pers.tile([P, NQS // P], F32)
            cent = pers.tile([C, D], F32)
            csq = pers.tile([C, D], F32)
            red_sb = pers.tile([C, D + 1], BF16)
            redr_sb = pers.tile([C, D + 1], BF16)
            b_row = pers.tile([1, C], F32)
            cnt_t = pers.tile([C, 1], F32)
            rc_t = pers.tile([C, 1], F32)
            bc2 = pers.tile([C, 1], F32)

            q_bf = dram.tile([NQS, D], BF16)            # natural layout
            q_tt = dram.tile([NSUP, DCH, P, SUP], BF16)  # transposed, tiled
            b_dram = dram.tile([1, C], F32)

            # ---------------- pass 1: normalize + transpose queries -------
            with tc.tile_pool(name="p1", bufs=1) as p1:
                for s in range(NSUP):
                    qts_sup = p1.tile([P, DCH, SUP], BF16, tag="qts", bufs=2)
                    for k in range(KSUB):
                        t = s * KSUB + k
                        qx_t = p1.tile([P, D], F32, tag="qx", bufs=3)
                        nc.scalar.dma_start(qx_t[:], qx_d[ts(t, P), :])
                        qc = p1.tile([P, D], F32, tag="qc", bufs=2)
                        nc.vector.scalar_tensor_tensor(
                            qc[:], qx_t[:], 1.0, mean_bc[:],
                            op0=OP.mult, op1=OP.subtract,
                        )
                        sq = p1.tile([P, D], F32, tag="sq", bufs=2)
                        nrm = p1.tile([P, 1], F32, tag="nrm", bufs=2)
                        nc.scalar.activation(sq[:], qc[:], AF.Square,
                                             accum_out=nrm[:])
                        rin = p1.tile([P, 1], F32, tag="rin", bufs=2)
                        nc.vector.reciprocal(rin[:], nrm[:])
                        rs = p1.tile([P, 1], F32, tag="rs", bufs=2)
                        nc.scalar.activation(rs[:], rin[:], AF.Sqrt)
                        qb = p1.tile([P, D], BF16, tag="qb", bufs=3)
                        nc.vector.tensor_scalar(
                            qb[:], qc[:], rs[:], None, op0=OP.mult
                        )
                        nc.sync.dma_start(q_bf[ts(t, P), :], qb[:])
                        # transpose on PE: 8 chunks of [128,128]
                        tp = ps_t.tile([P, DCH, P], BF16, tag="qtp")
                        for j in range(DCH):
                            nc.tensor.transpose(
                                tp[:, j, :], qb[:, ts(j, P)], identb_sb[:]
                            )
                        nc.scalar.copy(qts_sup[:, :, ts(k, P)], tp[:])
                    for j in range(DCH):
                        nc.sync.dma_start(q_tt[s, j], qts_sup[:, j, :])

            # ---------------- iteration + pred passes ----------------
            with (
                tc.tile_pool(name="pq", bufs=3) as pq,
                tc.tile_pool(name="pt", bufs=3) as pt,
                tc.tile_pool(name="pw", bufs=3) as pw,
                tc.tile_pool(name="py", bufs=2) as py,
            ):
                for it in range(n_iter + 1):
                    pred = it == n_iter
                    if not pred:
                        suma_ps = ps_a.tile([C, D], F32, tag="suma")
                        sumb_ps = ps_b.tile([C, 1], F32, tag="sumb")

                    for s in range(NSUP):
                        qT_sup = pt.tile([P, DCH, SUP], BF16, tag="qT")
                        for j in range(DCH):
                            nc.sync.dma_start(qT_sup[:, j, :], q_tt[s, j])
                        if not pred:
                            q_sup = pq.tile([P, KSUB, D], BF16, tag="q")
                            for k in range(KSUB):
                                nc.scalar.dma_start(
                                    q_sup[:, k, :],
                                    q_bf[ts(s * KSUB + k, P), :],
                                )
                        else:
                            yh_sup = py.tile([P, KSUB, C], F32, tag="yh")
                            for k in range(KSUB):
                                nc.scalar.dma_start(
                                    yh_sup[:, k, :],
                                    y1h_d[ts(s * KSUB + k, P), :],
                                )

                        for k in range(KSUB):
                            sc = ps_sc.tile([P, C], F32, tag="sc")
                            for j in range(DCH):
                                nc.tensor.matmul(
                                    sc[:],
                                    qT_sup[:, j, ts(k, P)],
                                    cT_sb[:, ts(j, C)],
                                    start=(j == 0),
                                    stop=(j == DCH - 1),
                                )
                            zt = pw.tile([P, C], F32, tag="zt")
                            nc.vector.scalar_tensor_tensor(
                                zt[:], sc[:], two_beta, Bb[:],
                                op0=OP.mult, op1=OP.subtract,
                            )
                            if not pred:
                                wt = pw.tile([P, C], BF16, tag="wt")
                                rsum = pw.tile([P, 1], F32, tag="rsum")
                                nc.scalar.activation(
                                    wt[:], zt[:], AF.Exp, accum_out=rsum[:]
                                )
                                rcp = pw.tile([P, 1], F32, tag="rcp")
                                nc.vector.reciprocal(rcp[:], rsum[:])
                                wn = pw.tile([P, C], BF16, tag="wn")
                                nc.scalar.mul(wn[:], wt[:], rcp[:])

                                first = (s == 0 and k == 0)
                                last = (s == NSUP - 1 and k == KSUB - 1)
                                nc.tensor.matmul(
                                    suma_ps[:, 0:512], wn[:],
                                    q_sup[:, k, 0:512],
                                    start=first, stop=last,
                                )
                                nc.tensor.matmul(
                                    suma_ps[:, 512:1024], wn[:],
                                    q_sup[:, k, 512:1024],
                                    start=first, stop=last,
                                )
                                nc.tensor.matmul(
                                    sumb_ps[:], wn[:], ones_sb[:],
                                    start=first, stop=last,
                                )
                            else:
                                zmax = pw.tile([P, 1], F32, tag="zmax")
                                nc.vector.reduce_max(
                                    zmax[:], zt[:], axis=mybir.AxisListType.X
                                )
                                zsc = pw.tile([P, C], F32, tag="zsc")
                                zy = pw.tile([P, 1], F32, tag="zy")
                                nc.vector.scalar_tensor_tensor(
                                    zsc[:], zt[:], 1.0, yh_sup[:, k, :],
                                    op0=OP.mult, op1=OP.mult,
                                    accum_out=zy[:],
                                )
                                t_idx = s * KSUB + k
                                nc.vector.scalar_tensor_tensor(
                                    matches[:, t_idx:t_idx + 1],
                                    zmax[:], 1.0, zy[:],
                                    op0=OP.mult, op1=OP.subtract,
                                )

                    if pred:
                        nc.sync.dma_start(acc_d[:], matches[:])
                        continue

                    # ------- iteration update: allreduce + new centroids -----
                    nc.scalar.copy(red_sb[:, 0:D], suma_ps[:])
                    nc.scalar.copy(red_sb[:, D:D + 1], sumb_ps[:])
                    cc_in = ccp.tile([C, D + 1], BF16, tag="cci")
                    cc_out = ccp.tile([C, D + 1], BF16, tag="cco",
                                      addr_space="Shared")
                    nc.scalar.dma_start(cc_in[:], red_sb[:])
                    nc.gpsimd.collective_compute(
                        "AllReduce",
                        OP.add,
                        replica_groups=[list(range(N_CORES))],
                        ins=[cc_in[:].opt()],
                        outs=[cc_out[:].opt()],
                    )
                    nc.scalar.dma_start(redr_sb[:], cc_out[:])

                    nc.vector.scalar_tensor_tensor(
                        cent[:], redr_sb[:, 0:D], 1.0, ssum_sb[:],
                        op0=OP.mult, op1=OP.add,
                    )
                    nc.vector.scalar_tensor_tensor(
                        cnt_t[:], redr_sb[:, D:D + 1], 1.0, scnt_sb[:],
                        op0=OP.mult, op1=OP.add,
                    )
                    nc.vector.reciprocal(rc_t[:], cnt_t[:])
                    nc.vector.tensor_scalar(
                        cent[:], cent[:], rc_t[:], None, op0=OP.mult
                    )
                    nc.vector.scalar_tensor_tensor(
                        csq[:], cent[:], float(beta), cent[:],
                        op0=OP.mult, op1=OP.mult,
                        accum_out=bc2[:],
                    )
                    for j in range(DCH):
                        ct_ps = ps_t.tile([P, C], F32, tag="ct")
                        nc.tensor.transpose(
                            ct_ps[:], cent[:, ts(j, P)], ident_sb[0:C, 0:C]
                        )
                        nc.scalar.copy(cT_sb[:, ts(j, C)], ct_ps[:])
                    b_ps = ps_t.tile([1, C], F32, tag="ct")
                    nc.tensor.transpose(b_ps[:], bc2[:], ident_sb[0:C, 0:C])
                    nc.scalar.copy(b_row[:], b_ps[:])
                    nc.scalar.dma_start(b_dram[:], b_row[:])
                    nc.scalar.dma_start(
                        Bb[:], b_dram[:].broadcast_to([P, C])
                    )

    nc.compile()
    _BUILD_CACHE[key] = nc
    return nc


def kernel(**inputs) -> np.ndarray:
    global LAST_EXEC_NS
    support_x = np.asarray(inputs["support_x"], dtype=np.float64)
    support_y = np.asarray(inputs["support_y"]).astype(np.int64)
    query_x = np.asarray(inputs["query_x"], dtype=np.float32)
    query_y = np.asarray(inputs["query_y"]).astype(np.int64)
    n_iter = int(inputs["n_iter"])
    beta = float(inputs["beta"])
    num_classes = int(inputs["num_classes"])
    assert num_classes == C and query_x.shape == (NQ, D)
    assert support_x.shape == (len(support_y), D)

    # ---- host: support-set preprocessing (tiny) ----
    mean = support_x.mean(axis=0)
    s = support_x - mean
    s /= np.linalg.norm(s, axis=1, keepdims=True)
    sup_sum = np.zeros((C, D), dtype=np.float64)
    np.add.at(sup_sum, support_y, s)
    sup_cnt = np.bincount(support_y, minlength=C).astype(np.float64)
    c0 = sup_sum / sup_cnt[:, None]
    b0 = beta * (c0 * c0).sum(axis=1)

    y1h = np.zeros((NQ, C), dtype=np.float32)
    y1h[np.arange(NQ), query_y] = 1.0

    nc = build(n_iter, beta)

    mean_f = mean.astype(np.float32)[None, :]
    ssum_f = sup_sum.astype(np.float32)
    scnt_f = sup_cnt.astype(np.float32)[:, None]
    # [P, DCH*C]: row p holds [c0T[j*P+p, c] for j in range(DCH) for c in range(C)]
    c0t = np.ascontiguousarray(
        c0.T.reshape(DCH, P, C).transpose(1, 0, 2).reshape(P, DCH * C)
    ).astype(ml_dtypes.bfloat16)
    b0_f = b0.astype(np.float32)[None, :]
    ident = np.eye(P, dtype=np.float32)
    identb = np.eye(P).astype(ml_dtypes.bfloat16)
    ones = np.ones((P, 1), dtype=ml_dtypes.bfloat16)

    in_maps = []
    for i in range(N_CORES):
        in_maps.append({
            "qx": np.ascontiguousarray(query_x[ts(i, NQS)]),
            "y1h": np.ascontiguousarray(y1h[ts(i, NQS)]),
            "mean": mean_f,
            "ssum": ssum_f,
            "scnt": scnt_f,
            "c0t": c0t,
            "b0": b0_f,
            "ident": ident,
            "identb": identb,
            "ones": ones,
        })

    trace = bool(int(os.environ.get("KM_TRACE", "0")))
    if trace:
        import profile_util
        results, outdir = profile_util.profile_run(nc, in_maps, N_CORES)
        print(f"profile dir: {outdir}")
        try:
            LAST_EXEC_NS, _ = profile_util.exec_time_ns(outdir, device=0)
        except Exception as e:
            print(f"profile post-processing failed: {e}")
    else:
        res = run_bass_kernel_spmd(
            nc, in_maps, core_ids=list(range(N_CORES))
        )
        results = res.results
    total = sum(int((r["acc_out"] == 0.0).sum()) for r in results)
    return np.float32(total / NQ)


# revision 17
# speedup vs baseline: 2.1044x; 1.0960x over previous
"""Soft-KMeans (retrieval_knn) Trainium2 kernel, 8-core SPMD.

Full inputs in, full output (scalar accuracy) out. Internally:
 - host: support-set preprocessing (tiny: 1000x1024), query sharding 8-way
 - device: query normalization, n_iter soft-kmeans iterations with an
   AllReduce of per-class weighted sums each iteration, final argmax
   accuracy count.

Data layout: normalized queries are stored bf16 in DRAM twice --
natural [NQS, D] (rhs of the w^T q matmul; per-128-row blocks are
address-sequential) and transposed pre-tiled [NSUP, D/128, 128, 512]
(lhsT of the q @ c^T scores matmul; per-(super, chunk) blocks are
address-sequential). The transpose itself runs once, on the PE during
pass 1 -- xbar transpose DMAs from HBM move 256B packets and measure
~80 GB/s, so they are avoided entirely in the iteration loop.
"""
import os
import numpy as np
import ml_dtypes

import concourse.bass as bass
import concourse.bacc as bacc
import concourse.mybir as mybir
import concourse.tile as tile
from concourse.bass_utils import run_bass_kernel_spmd

F32 = mybir.dt.float32
BF16 = mybir.dt.bfloat16
AF = mybir.ActivationFunctionType
OP = mybir.AluOpType

N_CORES = 8
NQ = 131072
D = 1024
C = 100
P = 128
NQS = NQ // N_CORES          # queries per core (16384)
KSUB = 4                     # 128-row sub-tiles per super-tile
SUP = P * KSUB               # 512 queries per super-tile
NSUP = NQS // SUP            # super-tiles per core (32)
DCH = D // P                 # d chunks (8)
RES = 6                      # super-tiles kept SBUF-resident (last RES)

_BUILD_CACHE = {}
LAST_EXEC_NS = None


def ts(i, n):
    return slice(i * n, (i + 1) * n)


def build(n_iter: int, beta: float):
    key = (n_iter, float(beta))
    if key in _BUILD_CACHE:
        return _BUILD_CACHE[key]

    two_beta = 2.0 * float(beta)

    nc = bacc.Bacc(None, target_bir_lowering=False, num_devices=N_CORES)

    qx_d = nc.dram_tensor("qx", [NQS, D], F32, kind="ExternalInput")
    y1h_d = nc.dram_tensor("y1h", [NQS, C], F32, kind="ExternalInput")
    mean_d = nc.dram_tensor("mean", [1, D], F32, kind="ExternalInput")
    ssum_d = nc.dram_tensor("ssum", [C, D], F32, kind="ExternalInput")
    scnt_d = nc.dram_tensor("scnt", [C, 1], F32, kind="ExternalInput")
    c0t_d = nc.dram_tensor("c0t", [P, DCH * C], BF16, kind="ExternalInput")
    b0_d = nc.dram_tensor("b0", [1, C], F32, kind="ExternalInput")
    ident_d = nc.dram_tensor("ident", [P, P], F32, kind="ExternalInput")
    identb_d = nc.dram_tensor("identb", [P, P], BF16, kind="ExternalInput")
    ones_d = nc.dram_tensor("ones", [P, 1], BF16, kind="ExternalInput")
    # zmax - z[y] per query; exact 0.0 <=> prediction correct (host counts)
    acc_d = nc.dram_tensor("acc_out", [P, NQS // P], F32, kind="ExternalOutput")

    with tile.TileContext(nc) as tc:
        with (
            tc.tile_pool(name="pers", bufs=1) as pers,
            tc.tile_pool(name="dram", bufs=1, space="DRAM") as dram,
            tc.tile_pool(name="ccp", bufs=2, space="DRAM") as ccp,
            tc.tile_pool(name="ps_sc", bufs=3, space="PSUM") as ps_sc,
            tc.tile_pool(name="ps_a", bufs=1, space="PSUM") as ps_a,
            tc.tile_pool(name="ps_b", bufs=1, space="PSUM") as ps_b,
            tc.tile_pool(name="ps_t", bufs=1, space="PSUM") as ps_t,
        ):
            # ---------------- persistent tiles ----------------
            mean_bc = pers.tile([P, D], F32)
            nc.sync.dma_start(mean_bc[:], mean_d[:].broadcast_to([P, D]))

            ssum_sb = pers.tile([C, D], F32)
            nc.sync.dma_start(ssum_sb[:], ssum_d[:])
            scnt_sb = pers.tile([C, 1], F32)
            nc.sync.dma_start(scnt_sb[:], scnt_d[:])

            cT_sb = pers.tile([P, DCH * C], BF16)
            nc.sync.dma_start(cT_sb[:], c0t_d[:])

            Bb = pers.tile([P, C], F32)
            nc.sync.dma_start(Bb[:], b0_d[:].broadcast_to([P, C]))

            ident_sb = pers.tile([P, P], F32)
            nc.sync.dma_start(ident_sb[:], ident_d[:])
            identb_sb = pers.tile([P, P], BF16)
            nc.sync.dma_start(identb_sb[:], identb_d[:])
            ones_sb = pers.tile([P, 1], BF16)
            nc.sync.dma_start(ones_sb[:], ones_d[:])

            matches = pers.tile([P, NQS // P], F32)
            cent = pers.tile([C, D], F32)
            csq = pers.tile([C, D], F32)
            red_sb = pers.tile([C, D + 1], BF16)
            redr_sb = pers.tile([C, D + 1], BF16)
            b_row = pers.tile([1, C], F32)
            cnt_t = pers.tile([C, 1], F32)
            rc_t = pers.tile([C, 1], F32)
            bc2 = pers.tile([C, 1], F32)

            # resident super-tiles (the last RES): both layouts live in SBUF
            qT_res = [pers.tile([P, DCH, SUP], BF16, name=f"qTres{r}")
                      for r in range(RES)]
            q_res = [pers.tile([P, KSUB, D], BF16, name=f"qres{r}")
                     for r in range(RES)]

            q_bf = dram.tile([NQS, D], BF16)            # natural layout
            q_tt = dram.tile([NSUP, DCH, P, SUP], BF16)  # transposed, tiled
            b_dram = dram.tile([1, C], F32)

            # ---------------- pass 1: normalize + transpose queries -------
            with tc.tile_pool(name="p1", bufs=1) as p1:
                for s in range(NSUP):
                    res = s - (NSUP - RES)  # >= 0 when resident
                    if res >= 0:
                        qts_sup = qT_res[res]
                    else:
                        qts_sup = p1.tile([P, DCH, SUP], BF16, tag="qts",
                                          bufs=2)
                    for k in range(KSUB):
                        t = s * KSUB + k
                        qx_t = p1.tile([P, D], F32, tag="qx", bufs=3)
                        nc.scalar.dma_start(qx_t[:], qx_d[ts(t, P), :])
                        qc = p1.tile([P, D], F32, tag="qc", bufs=2)
                        nc.vector.scalar_tensor_tensor(
                            qc[:], qx_t[:], 1.0, mean_bc[:],
                            op0=OP.mult, op1=OP.subtract,
                        )
                        nrm = p1.tile([P, 1], F32, tag="nrm", bufs=2)
                        # Square output overwrites the dead qx_t slot
                        nc.scalar.activation(qx_t[:], qc[:], AF.Square,
                                             accum_out=nrm[:])
                        rin = p1.tile([P, 1], F32, tag="rin", bufs=2)
                        nc.vector.reciprocal(rin[:], nrm[:])
                        rs = p1.tile([P, 1], F32, tag="rs", bufs=2)
                        nc.scalar.activation(rs[:], rin[:], AF.Sqrt)
                        if res >= 0:
                            qb = q_res[res][:, k, :]
                        else:
                            qb_t = p1.tile([P, D], BF16, tag="qb", bufs=3)
                            qb = qb_t[:]
                        nc.vector.tensor_scalar(
                            qb, qc[:], rs[:], None, op0=OP.mult
                        )
                        if res < 0:
                            nc.sync.dma_start(q_bf[ts(t, P), :], qb)
                        # transpose on PE: 8 chunks of [128,128]
                        tp = ps_sc.tile([P, DCH, P], BF16, tag="sc")
                        for j in range(DCH):
                            nc.tensor.transpose(
                                tp[:, j, :], qb[:, ts(j, P)], identb_sb[:]
                            )
                        nc.scalar.copy(qts_sup[:, :, ts(k, P)], tp[:])
                    if res < 0:
                        for j in range(DCH):
                            nc.sync.dma_start(q_tt[s, j], qts_sup[:, j, :])

            # ---------------- iteration + pred passes ----------------
            with (
                tc.tile_pool(name="pq", bufs=3) as pq,
                tc.tile_pool(name="pt", bufs=3) as pt,
                tc.tile_pool(name="pw", bufs=3) as pw,
                tc.tile_pool(name="py", bufs=2) as py,
            ):
                for it in range(n_iter + 1):
                    pred = it == n_iter
                    if not pred:
                        suma_ps = ps_a.tile([C, D], F32, tag="suma")
                        sumb_ps = ps_b.tile([C, 1], F32, tag="sumb")

                    for s in range(NSUP):
                        res = s - (NSUP - RES)
                        if res >= 0:
                            qT_sup = qT_res[res]
                            q_sup = q_res[res]
                        else:
                            qT_sup = pt.tile([P, DCH, SUP], BF16, tag="qT")
                            for j in range(DCH):
                                nc.sync.dma_start(qT_sup[:, j, :], q_tt[s, j])
                            if not pred:
                                q_sup = pq.tile([P, KSUB, D], BF16, tag="q")
                                for k in range(KSUB):
                                    nc.scalar.dma_start(
                                        q_sup[:, k, :],
                                        q_bf[ts(s * KSUB + k, P), :],
                                    )
                        if pred:
                            yh_sup = py.tile([P, KSUB, C], F32, tag="yh")
                            for k in range(KSUB):
                                nc.scalar.dma_start(
                                    yh_sup[:, k, :],
                                    y1h_d[ts(s * KSUB + k, P), :],
                                )

                        for k in range(KSUB):
                            sc = ps_sc.tile([P, C], F32, tag="sc")
                            for j in range(DCH):
                                nc.tensor.matmul(
                                    sc[:],
                                    qT_sup[:, j, ts(k, P)],
                                    cT_sb[:, ts(j, C)],
                                    start=(j == 0),
                                    stop=(j == DCH - 1),
                                )
                            zt = pw.tile([P, C], F32, tag="zt")
                            nc.vector.scalar_tensor_tensor(
                                zt[:], sc[:], two_beta, Bb[:],
                                op0=OP.mult, op1=OP.subtract,
                            )
                            if not pred:
                                wt = pw.tile([P, C], BF16, tag="wt")
                                rsum = pw.tile([P, 1], F32, tag="rsum")
                                nc.scalar.activation(
                                    wt[:], zt[:], AF.Exp, accum_out=rsum[:]
                                )
                                rcp = pw.tile([P, 1], F32, tag="rcp")
                                nc.vector.reciprocal(rcp[:], rsum[:])
                                wn = pw.tile([P, C], BF16, tag="wn")
                                nc.scalar.mul(wn[:], wt[:], rcp[:])

                                first = (s == 0 and k == 0)
                                last = (s == NSUP - 1 and k == KSUB - 1)
                                nc.tensor.matmul(
                                    suma_ps[:, 0:512], wn[:],
                                    q_sup[:, k, 0:512],
                                    start=first, stop=last,
                                )
                                nc.tensor.matmul(
                                    suma_ps[:, 512:1024], wn[:],
                                    q_sup[:, k, 512:1024],
                                    start=first, stop=last,
                                )
                                nc.tensor.matmul(
                                    sumb_ps[:], wn[:], ones_sb[:],
                                    start=first, stop=last,
                                )
                            else:
                                zmax = pw.tile([P, 1], F32, tag="zmax")
                                nc.vector.reduce_max(
                                    zmax[:], zt[:], axis=mybir.AxisListType.X
                                )
                                zsc = pw.tile([P, C], F32, tag="zsc")
                                zy = pw.tile([P, 1], F32, tag="zy")
                                nc.vector.scalar_tensor_tensor(
                                    zsc[:], zt[:], 1.0, yh_sup[:, k, :],
                                    op0=OP.mult, op1=OP.mult,
                                    accum_out=zy[:],
                                )
                                t_idx = s * KSUB + k
                                nc.vector.scalar_tensor_tensor(
                                    matches[:, t_idx:t_idx + 1],
                                    zmax[:], 1.0, zy[:],
                                    op0=OP.mult, op1=OP.subtract,
                                )

                    if pred:
                        nc.sync.dma_start(acc_d[:], matches[:])
                        continue

                    # ------- iteration update: allreduce + new centroids -----
                    nc.scalar.copy(red_sb[:, 0:D], suma_ps[:])
                    nc.scalar.copy(red_sb[:, D:D + 1], sumb_ps[:])
                    cc_in = ccp.tile([C, D + 1], BF16, tag="cci")
                    cc_out = ccp.tile([C, D + 1], BF16, tag="cco",
                                      addr_space="Shared")
                    nc.scalar.dma_start(cc_in[:], red_sb[:])
                    nc.gpsimd.collective_compute(
                        "AllReduce",
                        OP.add,
                        replica_groups=[list(range(N_CORES))],
                        ins=[cc_in[:].opt()],
                        outs=[cc_out[:].opt()],
                    )
                    nc.scalar.dma_start(redr_sb[:], cc_out[:])

                    nc.vector.scalar_tensor_tensor(
                        cent[:], redr_sb[:, 0:D], 1.0, ssum_sb[:],
                        op0=OP.mult, op1=OP.add,
                    )
                    nc.vector.scalar_tensor_tensor(
                        cnt_t[:], redr_sb[:, D:D + 1], 1.0, scnt_sb[:],
                        op0=OP.mult, op1=OP.add,
                    )
                    nc.vector.reciprocal(rc_t[:], cnt_t[:])
                    nc.vector.tensor_scalar(
                        cent[:], cent[:], rc_t[:], None, op0=OP.mult
                    )
                    nc.vector.scalar_tensor_tensor(
                        csq[:], cent[:], float(beta), cent[:],
                        op0=OP.mult, op1=OP.mult,
                        accum_out=bc2[:],
                    )
                    for j in range(DCH):
                        ct_ps = ps_t.tile([P, C], F32, tag="ct")
                        nc.tensor.transpose(
                            ct_ps[:], cent[:, ts(j, P)], ident_sb[0:C, 0:C]
                        )
                        nc.scalar.copy(cT_sb[:, ts(j, C)], ct_ps[:])
                    b_ps = ps_t.tile([1, C], F32, tag="ct")
                    nc.tensor.transpose(b_ps[:], bc2[:], ident_sb[0:C, 0:C])
                    nc.scalar.copy(b_row[:], b_ps[:])
                    nc.scalar.dma_start(b_dram[:], b_row[:])
                    nc.scalar.dma_start(
                        Bb[:], b_dram[:].broadcast_to([P, C])
                    )

    nc.compile()
    _BUILD_CACHE[key] = nc
    return nc


def kernel(**inputs) -> np.ndarray:
    global LAST_EXEC_NS
    support_x = np.asarray(inputs["support_x"], dtype=np.float64)
    support_y = np.asarray(inputs["support_y"]).astype(np.int64)
    query_x = np.asarray(inputs["query_x"], dtype=np.float32)
    query_y = np.asarray(inputs["query_y"]).astype(np.int64)
    n_iter = int(inputs["n_iter"])
    beta = float(inputs["beta"])
    num_classes = int(inputs["num_classes"])
    assert num_classes == C and query_x.shape == (NQ, D)
    assert support_x.shape == (len(support_y), D)

    # ---- host: support-set preprocessing (tiny) ----
    mean = support_x.mean(axis=0)
    s = support_x - mean
    s /= np.linalg.norm(s, axis=1, keepdims=True)
    sup_sum = np.zeros((C, D), dtype=np.float64)
    np.add.at(sup_sum, support_y, s)
    sup_cnt = np.bincount(support_y, minlength=C).astype(np.float64)
    c0 = sup_sum / sup_cnt[:, None]
    b0 = beta * (c0 * c0).sum(axis=1)

    y1h = np.zeros((NQ, C), dtype=np.float32)
    y1h[np.arange(NQ), query_y] = 1.0

    nc = build(n_iter, beta)

    mean_f = mean.astype(np.float32)[None, :]
    ssum_f = sup_sum.astype(np.float32)
    scnt_f = sup_cnt.astype(np.float32)[:, None]
    # [P, DCH*C]: row p holds [c0T[j*P+p, c] for j in range(DCH) for c in range(C)]
    c0t = np.ascontiguousarray(
        c0.T.reshape(DCH, P, C).transpose(1, 0, 2).reshape(P, DCH * C)
    ).astype(ml_dtypes.bfloat16)
    b0_f = b0.astype(np.float32)[None, :]
    ident = np.eye(P, dtype=np.float32)
    identb = np.eye(P).astype(ml_dtypes.bfloat16)
    ones = np.ones((P, 1), dtype=ml_dtypes.bfloat16)

    in_maps = []
    for i in range(N_CORES):
        in_maps.append({
            "qx": np.ascontiguousarray(query_x[ts(i, NQS)]),
            "y1h": np.ascontiguousarray(y1h[ts(i, NQS)]),
            "mean": mean_f,
            "ssum": ssum_f,
            "scnt": scnt_f,
            "c0t": c0t,
            "b0": b0_f,
            "ident": ident,
            "identb": identb,
            "ones": ones,
        })

    trace = bool(int(os.environ.get("KM_TRACE", "0")))
    if trace:
        import profile_util
        results, outdir = profile_util.profile_run(nc, in_maps, N_CORES)
        print(f"profile dir: {outdir}")
        try:
            LAST_EXEC_NS, _ = profile_util.exec_time_ns(outdir, device=0)
        except Exception as e:
            print(f"profile post-processing failed: {e}")
    else:
        res = run_bass_kernel_spmd(
            nc, in_maps, core_ids=list(range(N_CORES))
        )
        results = res.results
    total = sum(int((r["acc_out"] == 0.0).sum()) for r in results)
    return np.float32(total / NQ)


# revision 19
# speedup vs baseline: 2.1185x; 1.0067x over previous
"""Soft-KMeans (retrieval_knn) Trainium2 kernel, 8-core SPMD.

Full inputs in, full output (scalar accuracy) out. Internally:
 - host: support-set preprocessing (tiny: 1000x1024), query sharding 8-way
 - device: query normalization, n_iter soft-kmeans iterations with an
   AllReduce of per-class weighted sums each iteration, final argmax
   accuracy count.

Data layout: normalized queries are stored bf16 in DRAM twice --
natural [NQS, D] (rhs of the w^T q matmul; per-128-row blocks are
address-sequential) and transposed pre-tiled [NSUP, D/128, 128, 512]
(lhsT of the q @ c^T scores matmul; per-(super, chunk) blocks are
address-sequential). The transpose itself runs once, on the PE during
pass 1 -- xbar transpose DMAs from HBM move 256B packets and measure
~80 GB/s, so they are avoided entirely in the iteration loop.
"""
import os
import numpy as np
import ml_dtypes

import concourse.bass as bass
import concourse.bacc as bacc
import concourse.mybir as mybir
import concourse.tile as tile
from concourse.bass_utils import run_bass_kernel_spmd

F32 = mybir.dt.float32
BF16 = mybir.dt.bfloat16
AF = mybir.ActivationFunctionType
OP = mybir.AluOpType

N_CORES = 8
NQ = 131072
D = 1024
C = 100
P = 128
NQS = NQ // N_CORES          # queries per core (16384)
KSUB = 4                     # 128-row sub-tiles per super-tile
SUP = P * KSUB               # 512 queries per super-tile
NSUP = NQS // SUP            # super-tiles per core (32)
DCH = D // P                 # d chunks (8)
RES = 6                      # super-tiles kept SBUF-resident (last RES)

_BUILD_CACHE = {}
LAST_EXEC_NS = None


def ts(i, n):
    return slice(i * n, (i + 1) * n)


def build(n_iter: int, beta: float):
    key = (n_iter, float(beta))
    if key in _BUILD_CACHE:
        return _BUILD_CACHE[key]

    two_beta = 2.0 * float(beta)

    nc = bacc.Bacc(None, target_bir_lowering=False, num_devices=N_CORES)

    qx_d = nc.dram_tensor("qx", [NQS, D], F32, kind="ExternalInput")
    y1h_d = nc.dram_tensor("y1h", [NQS, C], F32, kind="ExternalInput")
    mean_d = nc.dram_tensor("mean", [1, D], F32, kind="ExternalInput")
    ssum_d = nc.dram_tensor("ssum", [C, D], F32, kind="ExternalInput")
    scnt_d = nc.dram_tensor("scnt", [C, 1], F32, kind="ExternalInput")
    c0t_d = nc.dram_tensor("c0t", [P, DCH * C], BF16, kind="ExternalInput")
    b0_d = nc.dram_tensor("b0", [1, C], F32, kind="ExternalInput")
    ident_d = nc.dram_tensor("ident", [P, P], F32, kind="ExternalInput")
    identb_d = nc.dram_tensor("identb", [P, P], BF16, kind="ExternalInput")
    ones_d = nc.dram_tensor("ones", [P, 1], BF16, kind="ExternalInput")
    # zmax - z[y] per query; exact 0.0 <=> prediction correct (host counts)
    acc_d = nc.dram_tensor("acc_out", [P, NQS // P], F32, kind="ExternalOutput")

    with tile.TileContext(nc) as tc:
        with (
            tc.tile_pool(name="pers", bufs=1) as pers,
            tc.tile_pool(name="dram", bufs=1, space="DRAM") as dram,
            tc.tile_pool(name="ccp", bufs=2, space="DRAM") as ccp,
            tc.tile_pool(name="ps_sc", bufs=3, space="PSUM") as ps_sc,
            tc.tile_pool(name="ps_a", bufs=1, space="PSUM") as ps_a,
            tc.tile_pool(name="ps_b", bufs=1, space="PSUM") as ps_b,
            tc.tile_pool(name="ps_t", bufs=1, space="PSUM") as ps_t,
        ):
            # ---------------- persistent tiles ----------------
            mean_bc = pers.tile([P, D], F32)
            nc.sync.dma_start(mean_bc[:], mean_d[:].broadcast_to([P, D]))

            ssum_sb = pers.tile([C, D], F32)
            nc.sync.dma_start(ssum_sb[:], ssum_d[:])
            scnt_sb = pers.tile([C, 1], F32)
            nc.sync.dma_start(scnt_sb[:], scnt_d[:])

            cT_sb = pers.tile([P, DCH * C], BF16)
            nc.sync.dma_start(cT_sb[:], c0t_d[:])

            Bb = pers.tile([P, C], F32)
            nc.sync.dma_start(Bb[:], b0_d[:].broadcast_to([P, C]))

            ident_sb = pers.tile([P, P], F32)
            nc.sync.dma_start(ident_sb[:], ident_d[:])
            identb_sb = pers.tile([P, P], BF16)
            nc.sync.dma_start(identb_sb[:], identb_d[:])
            ones_sb = pers.tile([P, 1], BF16)
            nc.sync.dma_start(ones_sb[:], ones_d[:])

            matches = pers.tile([P, NQS // P], F32)
            cent = pers.tile([C, D], F32)
            csq = pers.tile([C, D], F32)
            red_sb = pers.tile([C, D + 1], BF16)
            redr_sb = pers.tile([C, D + 1], BF16)
            b_row = pers.tile([1, C], F32)
            cnt_t = pers.tile([C, 1], F32)
            rc_t = pers.tile([C, 1], F32)
            bc2 = pers.tile([C, 1], F32)

            # resident super-tiles (the last RES): both layouts live in SBUF
            qT_res = [pers.tile([P, DCH, SUP], BF16, name=f"qTres{r}")
                      for r in range(RES)]
            q_res = [pers.tile([P, KSUB, D], BF16, name=f"qres{r}")
                     for r in range(RES)]

            q_bf = dram.tile([NQS, D], BF16)            # natural layout
            q_tt = dram.tile([NSUP, DCH, P, SUP], BF16)  # transposed, tiled
            b_dram = dram.tile([1, C], F32)

            # ---------------- pass 1: normalize + transpose queries -------
            with tc.tile_pool(name="p1", bufs=1) as p1:
                for s in range(NSUP):
                    res = s - (NSUP - RES)  # >= 0 when resident
                    if res >= 0:
                        qts_sup = qT_res[res]
                    else:
                        qts_sup = p1.tile([P, DCH, SUP], BF16, tag="qts",
                                          bufs=2)
                    for k in range(KSUB):
                        t = s * KSUB + k
                        qx_t = p1.tile([P, D], F32, tag="qx", bufs=3)
                        nc.scalar.dma_start(qx_t[:], qx_d[ts(t, P), :])
                        qc = p1.tile([P, D], F32, tag="qc", bufs=3)
                        nc.vector.scalar_tensor_tensor(
                            qc[:], qx_t[:], 1.0, mean_bc[:],
                            op0=OP.mult, op1=OP.subtract,
                        )
                        nrm = p1.tile([P, 1], F32, tag="nrm", bufs=4)
                        # Square output overwrites the dead qx_t slot
                        nc.scalar.activation(qx_t[:], qc[:], AF.Square,
                                             accum_out=nrm[:])
                        rin = p1.tile([P, 1], F32, tag="rin", bufs=4)
                        nc.vector.reciprocal(rin[:], nrm[:])
                        rs = p1.tile([P, 1], F32, tag="rs", bufs=4)
                        nc.scalar.activation(rs[:], rin[:], AF.Sqrt)
                        if res >= 0:
                            qb = q_res[res][:, k, :]
                        else:
                            qb_t = p1.tile([P, D], BF16, tag="qb", bufs=3)
                            qb = qb_t[:]
                        nc.vector.tensor_scalar(
                            qb, qc[:], rs[:], None, op0=OP.mult
                        )
                        if res < 0:
                            nc.sync.dma_start(q_bf[ts(t, P), :], qb)
                        # transpose on PE: 8 chunks of [128,128]
                        tp = ps_sc.tile([P, DCH, P], BF16, tag="sc")
                        for j in range(DCH):
                            nc.tensor.transpose(
                                tp[:, j, :], qb[:, ts(j, P)], identb_sb[:]
                            )
                        nc.scalar.copy(qts_sup[:, :, ts(k, P)], tp[:])
                    if res < 0:
                        for j in range(DCH):
                            nc.sync.dma_start(q_tt[s, j], qts_sup[:, j, :])

            # ---------------- iteration + pred passes ----------------
            with (
                tc.tile_pool(name="pq", bufs=3) as pq,
                tc.tile_pool(name="pt", bufs=3) as pt,
                tc.tile_pool(name="pw", bufs=3) as pw,
                tc.tile_pool(name="py", bufs=2) as py,
            ):
                for it in range(n_iter + 1):
                    pred = it == n_iter
                    if not pred:
                        suma_ps = ps_a.tile([C, D], F32, tag="suma")
                        sumb_ps = ps_b.tile([C, 1], F32, tag="sumb")

                    for s in range(NSUP):
                        res = s - (NSUP - RES)
                        if res >= 0:
                            qT_sup = qT_res[res]
                            q_sup = q_res[res]
                        else:
                            qT_sup = pt.tile([P, DCH, SUP], BF16, tag="qT")
                            for j in range(DCH):
                                nc.sync.dma_start(qT_sup[:, j, :], q_tt[s, j])
                            if not pred:
                                q_sup = pq.tile([P, KSUB, D], BF16, tag="q")
                                for k in range(KSUB):
                                    nc.scalar.dma_start(
                                        q_sup[:, k, :],
                                        q_bf[ts(s * KSUB + k, P), :],
                                    )
                        if pred:
                            yh_sup = py.tile([P, KSUB, C], F32, tag="yh")
                            for k in range(KSUB):
                                nc.scalar.dma_start(
                                    yh_sup[:, k, :],
                                    y1h_d[ts(s * KSUB + k, P), :],
                                )

                        for k in range(KSUB):
                            sc = ps_sc.tile([P, C], F32, tag="sc")
                            for j in range(DCH):
                                nc.tensor.matmul(
                                    sc[:],
                                    qT_sup[:, j, ts(k, P)],
                                    cT_sb[:, ts(j, C)],
                                    start=(j == 0),
                                    stop=(j == DCH - 1),
                                )
                            zt = pw.tile([P, C], F32, tag="zt")
                            nc.vector.scalar_tensor_tensor(
                                zt[:], sc[:], two_beta, Bb[:],
                                op0=OP.mult, op1=OP.subtract,
                            )
                            if not pred:
                                wt = pw.tile([P, C], BF16, tag="wt")
                                rsum = pw.tile([P, 1], F32, tag="rsum")
                                nc.scalar.activation(
                                    wt[:], zt[:], AF.Exp, accum_out=rsum[:]
                                )
                                rcp = pw.tile([P, 1], F32, tag="rcp")
                                nc.vector.reciprocal(rcp[:], rsum[:])
                                wn = pw.tile([P, C], BF16, tag="wn")
                                nc.scalar.mul(wn[:], wt[:], rcp[:])

                                first = (s == 0 and k == 0)
                                last = (s == NSUP - 1 and k == KSUB - 1)
                                nc.tensor.matmul(
                                    suma_ps[:, 0:512], wn[:],
                                    q_sup[:, k, 0:512],
                                    start=first, stop=last,
                                )
                                nc.tensor.matmul(
                                    suma_ps[:, 512:1024], wn[:],
                                    q_sup[:, k, 512:1024],
                                    start=first, stop=last,
                                )
                                nc.tensor.matmul(
                                    sumb_ps[:], wn[:], ones_sb[:],
                                    start=first, stop=last,
                                )
                            else:
                                zmax = pw.tile([P, 1], F32, tag="zmax")
                                nc.vector.reduce_max(
                                    zmax[:], zt[:], axis=mybir.AxisListType.X
                                )
                                zsc = pw.tile([P, C], F32, tag="zsc")
                                zy = pw.tile([P, 1], F32, tag="zy")
                                nc.vector.scalar_tensor_tensor(
                                    zsc[:], zt[:], 1.0, yh_sup[:, k, :],
                                    op0=OP.mult, op1=OP.mult,
                                    accum_out=zy[:],
                                )
                                t_idx = s * KSUB + k
                                nc.vector.scalar_tensor_tensor(
                                    matches[:, t_idx:t_idx + 1],
                                    zmax[:], 1.0, zy[:],
                                    op0=OP.mult, op1=OP.subtract,
                                )

                    if pred:
                        nc.sync.dma_start(acc_d[:], matches[:])
                        continue

                    # ------- iteration update: allreduce + new centroids -----
                    nc.scalar.copy(red_sb[:, 0:D], suma_ps[:])
                    nc.scalar.copy(red_sb[:, D:D + 1], sumb_ps[:])
                    cc_in = ccp.tile([C, D + 1], BF16, tag="cci")
                    cc_out = ccp.tile([C, D + 1], BF16, tag="cco",
                                      addr_space="Shared")
                    nc.scalar.dma_start(cc_in[:], red_sb[:])
                    nc.gpsimd.collective_compute(
                        "AllReduce",
                        OP.add,
                        replica_groups=[list(range(N_CORES))],
                        ins=[cc_in[:].opt()],
                        outs=[cc_out[:].opt()],
                    )
                    nc.scalar.dma_start(redr_sb[:], cc_out[:])

                    nc.vector.scalar_tensor_tensor(
                        cent[:], redr_sb[:, 0:D], 1.0, ssum_sb[:],
                        op0=OP.mult, op1=OP.add,
                    )
                    nc.vector.scalar_tensor_tensor(
                        cnt_t[:], redr_sb[:, D:D + 1], 1.0, scnt_sb[:],
                        op0=OP.mult, op1=OP.add,
                    )
                    nc.vector.reciprocal(rc_t[:], cnt_t[:])
                    nc.vector.tensor_scalar(
                        cent[:], cent[:], rc_t[:], None, op0=OP.mult
                    )
                    nc.vector.scalar_tensor_tensor(
                        csq[:], cent[:], float(beta), cent[:],
                        op0=OP.mult, op1=OP.mult,
                        accum_out=bc2[:],
                    )
                    # bias row first so its DMA round-trip overlaps the
                    # centroid transposes below
                    b_ps = ps_t.tile([1, C], F32, tag="ct", bufs=2)
                    nc.tensor.transpose(b_ps[:], bc2[:], ident_sb[0:C, 0:C])
                    nc.scalar.copy(b_row[:], b_ps[:])
                    nc.scalar.dma_start(b_dram[:], b_row[:])
                    nc.scalar.dma_start(
                        Bb[:], b_dram[:].broadcast_to([P, C])
                    )
                    for j in range(DCH):
                        ct_ps = ps_t.tile([P, C], F32, tag="ct", bufs=2)
                        nc.tensor.transpose(
                            ct_ps[:], cent[:, ts(j, P)], ident_sb[0:C, 0:C]
                        )
                        nc.scalar.copy(cT_sb[:, ts(j, C)], ct_ps[:])

    nc.compile()
    _BUILD_CACHE[key] = nc
    return nc


def kernel(**inputs) -> np.ndarray:
    global LAST_EXEC_NS
    support_x = np.asarray(inputs["support_x"], dtype=np.float64)
    support_y = np.asarray(inputs["support_y"]).astype(np.int64)
    query_x = np.asarray(inputs["query_x"], dtype=np.float32)
    query_y = np.asarray(inputs["query_y"]).astype(np.int64)
    n_iter = int(inputs["n_iter"])
    beta = float(inputs["beta"])
    num_classes = int(inputs["num_classes"])
    assert num_classes == C and query_x.shape == (NQ, D)
    assert support_x.shape == (len(support_y), D)

    # ---- host: support-set preprocessing (tiny) ----
    mean = support_x.mean(axis=0)
    s = support_x - mean
    s /= np.linalg.norm(s, axis=1, keepdims=True)
    sup_sum = np.zeros((C, D), dtype=np.float64)
    np.add.at(sup_sum, support_y, s)
    sup_cnt = np.bincount(support_y, minlength=C).astype(np.float64)
    c0 = sup_sum / sup_cnt[:, None]
    b0 = beta * (c0 * c0).sum(axis=1)

    y1h = np.zeros((NQ, C), dtype=np.float32)
    y1h[np.arange(NQ), query_y] = 1.0

    nc = build(n_iter, beta)

    mean_f = mean.astype(np.float32)[None, :]
    ssum_f = sup_sum.astype(np.float32)
    scnt_f = sup_cnt.astype(np.float32)[:, None]
    # [P, DCH*C]: row p holds [c0T[j*P+p, c] for j in range(DCH) for c in range(C)]
    c0t = np.ascontiguousarray(
        c0.T.reshape(DCH, P, C).transpose(1, 0, 2).reshape(P, DCH * C)
    ).astype(ml_dtypes.bfloat16)
    b0_f = b0.astype(np.float32)[None, :]
    ident = np.eye(P, dtype=np.float32)
    identb = np.eye(P).astype(ml_dtypes.bfloat16)
    ones = np.ones((P, 1), dtype=ml_dtypes.bfloat16)

    in_maps = []
    for i in range(N_CORES):
        in_maps.append({
            "qx": np.ascontiguousarray(query_x[ts(i, NQS)]),
            "y1h": np.ascontiguousarray(y1h[ts(i, NQS)]),
            "mean": mean_f,
            "ssum": ssum_f,
            "scnt": scnt_f,
            "c0t": c0t,
            "b0": b0_f,
            "ident": ident,
            "identb": identb,
            "ones": ones,
        })

    trace = bool(int(os.environ.get("KM_TRACE", "0")))
    if trace:
        import profile_util
        results, outdir = profile_util.profile_run(nc, in_maps, N_CORES)
        print(f"profile dir: {outdir}")
        try:
            LAST_EXEC_NS, _ = profile_util.exec_time_ns(outdir, device=0)
        except Exception as e:
            print(f"profile post-processing failed: {e}")
    else:
        res = run_bass_kernel_spmd(
            nc, in_maps, core_ids=list(range(N_CORES))
        )
        results = res.results
    total = sum(int((r["acc_out"] == 0.0).sum()) for r in results)
    return np.float32(total / NQ)
